# revision 9
# baseline (speedup 1.0000x reference)
"""Trainium2 Bass kernel for nn_DAWN_41549513621652.

Strategy (8 NeuronCores, single chip, no cross-core collectives):
  The model's heavy compute is dense matmul (attention, Wo, memory WV,
  lm_head). The glue (layernorm, the 512-step SSM scan, routing softmax,
  and the DMA-bound neuron-pool contractions nw@{comp,EQ,EK,EV}) is tiny
  FLOP-wise and runs on host between device launches; host also performs
  the cross-core reductions (summing Wo partials) so the device programs
  need no collectives.

  5 device launches per call:
    A (x2): circuit module, head-sharded — core c owns heads {2c, 2c+1}
            for both batch elements; outputs per-core Wo partials.
    C (x2): memory module, token-sharded — core c owns 128 tokens
            (b=c//4, s in [128*(c%4), ...)); exact top-16 via DVE
            max8/match_replace threshold, dense masked-softmax, PE WV.
    D (x1): lm_head, vocab-sharded — core c owns a 4096-wide slice of the
            zero-padded 32768 vocab.

  Everything is fp32: the memory module's top-16 selection has score gaps
  down to 7e-9, so any lower precision upstream flips selections vs the
  reference.
"""

import numpy as np

import concourse.bass as bass
import concourse.bacc as bacc
import concourse.mybir as mybir
import concourse.tile as tile
from concourse.bass_utils import run_bass_kernel_spmd
from concourse.masks import make_identity

F32 = mybir.dt.float32

# model dims (hardcoded per problem spec)
L, D, H, R, NC, NK, KK, SD, V, B, S = 2, 1024, 16, 128, 64, 1024, 16, 64, 32000, 2, 512
DH = D // H          # 64
T = B * S            # 1024
N_CORES = 8
VP = 32768           # padded vocab
VSL = VP // N_CORES  # 4096 per-core vocab slice
DT = D // 128        # 8 d-tiles
NEG = -1e30


# ---------------------------------------------------------------- device programs


def _build_A(n_iter: int = 1):
    """Circuit module. Per-core inputs:
      xnT  [D, T]    d-major normalized activations (cols = b*S+s)
      sc   [B, D, R] dynamic compress basis (host: nw@comp)
      eqs/eks/evs [B, R, 128]  expansion slices for this core's 2 heads
      woT  [128, D]  o_w.T rows for this core's d_in slice
      tri  [128, 128] upper-tri (incl diag) causal mask for scoresT layout
    Output:
      part [B, D, S] Wo partial, d-major
    """
    nc = bacc.Bacc("TRN2", target_bir_lowering=False, debug=False,
                   num_devices=N_CORES)
    xnT_d = nc.dram_tensor("xnT", [D, T], F32, kind="ExternalInput")
    sc_d = nc.dram_tensor("sc", [B, D, R], F32, kind="ExternalInput")
    eqs_d = nc.dram_tensor("eqs", [B, R, 128], F32, kind="ExternalInput")
    eks_d = nc.dram_tensor("eks", [B, R, 128], F32, kind="ExternalInput")
    evs_d = nc.dram_tensor("evs", [B, R, 128], F32, kind="ExternalInput")
    woT_d = nc.dram_tensor("woT", [128, D], F32, kind="ExternalInput")
    tri_d = nc.dram_tensor("tri", [128, 128], F32, kind="ExternalInput")
    part_d = nc.dram_tensor("part", [B, D, S], F32, kind="ExternalOutput")

    with tile.TileContext(nc) as tc:
        with (
            tc.tile_pool(name="big", bufs=1) as big,
            tc.tile_pool(name="work", bufs=2) as work,
            tc.tile_pool(name="small", bufs=2) as small,
            tc.tile_pool(name="ps", bufs=2, space="PSUM") as ps,
            tc.tile_pool(name="ps1", bufs=2, space="PSUM") as ps1,
            tc.tile_pool(name="out", bufs=3) as outp,
        ):
            def body(_it):
                xn = big.tile([128, DT, T], F32, tag="xn")
                nc.sync.dma_start(xn[:], xnT_d.ap().rearrange("(dt p) t -> p dt t", p=128))
                sc = big.tile([128, B, DT, R], F32, tag="sc")
                nc.sync.dma_start(sc[:], sc_d.ap().rearrange("b (dt p) r -> p b dt r", p=128))
                eq = big.tile([128, B, 128], F32, tag="eq")
                ek = big.tile([128, B, 128], F32, tag="ek")
                ev = big.tile([128, B, 128], F32, tag="ev")
                nc.sync.dma_start(eq[:], eqs_d.ap().rearrange("b r e -> r b e"))
                nc.sync.dma_start(ek[:], eks_d.ap().rearrange("b r e -> r b e"))
                nc.sync.dma_start(ev[:], evs_d.ap().rearrange("b r e -> r b e"))
                wo = big.tile([128, D], F32, tag="wo")
                nc.sync.dma_start(wo[:], woT_d.ap())
                tri = big.tile([128, 128], F32, tag="tri")
                nc.sync.dma_start(tri[:], tri_d.ap())
                ones = big.tile([128, 1], F32, tag="ones")
                nc.vector.memset(ones[:], 1.0)
                ones_row = big.tile([1, 64], F32, tag="ones_row")
                nc.vector.memset(ones_row[:], 1.0)

                # hT[b] [R=128, S] = sc[b].T @ xnT[b]
                h = big.tile([128, B, S], F32, tag="h")
                for b in range(B):
                    hp = ps.tile([128, S], F32, tag="mm")
                    for dt in range(DT):
                        nc.tensor.matmul(hp[:], sc[:, b, dt, :], xn[:, dt, b * S:(b + 1) * S],
                                         start=(dt == 0), stop=(dt == DT - 1))
                    nc.vector.tensor_copy(h[:, b, :], hp[:])

                # QT/KT [128(dh2), B, S]; V token-major [128(tok), B, 4, 128(dh2)]
                qt = big.tile([128, B, S], F32, tag="qt")
                kt_ = big.tile([128, B, S], F32, tag="kt")
                vt = big.tile([128, B, 4, 128], F32, tag="vt")
                for b in range(B):
                    qp = ps.tile([128, S], F32, tag="mm")
                    nc.tensor.matmul(qp[:], eq[:, b, :], h[:, b, :])
                    nc.vector.tensor_copy(qt[:, b, :], qp[:])
                    kp = ps.tile([128, S], F32, tag="mm")
                    nc.tensor.matmul(kp[:], ek[:, b, :], h[:, b, :])
                    nc.vector.tensor_copy(kt_[:, b, :], kp[:])
                    for st in range(4):
                        vp = ps1.tile([128, 512], F32, tag="aux1", name="vp")[:, :128]
                        nc.tensor.matmul(vp[:], h[:, b, st * 128:(st + 1) * 128], ev[:, b, :])
                        nc.vector.tensor_copy(vt[:, b, st, :], vp[:])

                # attention per (b, head-in-core)
                att = big.tile([128, B, S], F32, tag="att")  # [d_in(2*64), b, q]
                for b in range(B):
                    for hh in range(2):
                        p0 = 64 * hh
                        et = work.tile([128, 4, S], F32, tag="et")
                        for kt in range(4):
                            q0 = 128 * kt
                            sp = ps.tile([128, S], F32, tag="sp")
                            nc.tensor.matmul(
                                sp[:, q0:S],
                                kt_[p0:p0 + 64, b, kt * 128:(kt + 1) * 128],
                                qt[p0:p0 + 64, b, q0:S])
                            # e = exp(s / sqrt(DH)), causal-masked on the diagonal block
                            nc.scalar.activation(et[:, kt, q0:S], sp[:, q0:S],
                                                 mybir.ActivationFunctionType.Exp,
                                                 scale=float(1.0 / np.sqrt(DH)))
                            nc.vector.tensor_mul(et[:, kt, q0:q0 + 128],
                                                 et[:, kt, q0:q0 + 128], tri[:])
                        # Z[q] = sum_k e[k,q] via ones-matmul; then 1/Z
                        zp = ps1.tile([128, S], F32, tag="aux1", name="zp")[:1, :]
                        for kt in range(4):
                            nc.tensor.matmul(zp[:, 128 * kt:S], ones[:],
                                             et[:, kt, 128 * kt:S],
                                             start=(kt == 0), stop=(kt == 3))
                        zr = small.tile([1, S], F32, tag="zr")
                        nc.vector.reciprocal(zr[:], zp[:])
                        zbp = ps1.tile([128, S], F32, tag="aux1", name="zbp")[:64, :]
                        nc.tensor.matmul(zbp[:], ones_row[:], zr[:])
                        zb = small.tile([64, S], F32, tag="zb")
                        nc.vector.tensor_copy(zb[:], zbp[:])
                        # out_hT [64(dh), S] = sum_k V[k,dh].T-form @ e[k,q]
                        op_full = ps.tile([128, S], F32, tag="op", name="op")
                        op = op_full[:64, :]
                        for kt in range(4):
                            nc.tensor.matmul(op[:, 128 * kt:S],
                                             vt[:, b, kt, p0:p0 + 64],
                                             et[:, kt, 128 * kt:S],
                                             start=(kt == 0), stop=(kt == 3))
                        nc.vector.tensor_mul(att[p0:p0 + 64, b, :], op[:], zb[:])

                # Wo partial: part[b].T [d_out, S] = woT.T @ att[b]
                for b in range(B):
                    for mt in range(DT):
                        wp = ps.tile([128, S], F32, tag="mm")
                        nc.tensor.matmul(wp[:], wo[:, mt * 128:(mt + 1) * 128], att[:, b, :])
                        ot = outp.tile([128, S], F32, tag="ot")
                        nc.vector.tensor_copy(ot[:], wp[:])
                        nc.sync.dma_start(
                            part_d.ap()[b, mt * 128:(mt + 1) * 128, :], ot[:])

            if n_iter == 1:
                body(0)
            else:
                with tc.For_i(0, n_iter, 1) as it:
                    body(it)
    nc.compile()
    return nc


def _build_C(n_iter: int = 1):
    """Memory module, token-sharded (128 tokens per core). Inputs:
      xnTs [D, 128]  d-major xn columns for this core's tokens
      scb  [D, R]    compress basis for this core's batch element
      kKT  [R, NK]   knowledge_K.T
      kV   [NK, D]
    Output: mo [128, D] memory output rows for this core's tokens."""
    nc = bacc.Bacc("TRN2", target_bir_lowering=False, debug=False,
                   num_devices=N_CORES)
    xn_d = nc.dram_tensor("xnTs", [D, 128], F32, kind="ExternalInput")
    sc_d = nc.dram_tensor("scb", [D, R], F32, kind="ExternalInput")
    kk_d = nc.dram_tensor("kKT", [R, NK], F32, kind="ExternalInput")
    kv_d = nc.dram_tensor("kV", [NK, D], F32, kind="ExternalInput")
    mo_d = nc.dram_tensor("mo", [128, D], F32, kind="ExternalOutput")
    NT = NK // 128  # 8
    inv_sqrt_r = float(1.0 / np.sqrt(R))

    with tile.TileContext(nc) as tc:
        with (
            tc.tile_pool(name="big", bufs=1) as big,
            tc.tile_pool(name="work", bufs=2) as work,
            tc.tile_pool(name="ps", bufs=2, space="PSUM") as ps,
            tc.tile_pool(name="ps1", bufs=2, space="PSUM") as ps1,
        ):
            def body(_it):
                xn = big.tile([128, DT, 128], F32, tag="xn")
                nc.sync.dma_start(xn[:], xn_d.ap().rearrange("(dt p) t -> p dt t", p=128))
                sc = big.tile([128, DT, R], F32, tag="sc")
                nc.sync.dma_start(sc[:], sc_d.ap().rearrange("(dt p) r -> p dt r", p=128))
                kk = big.tile([128, NK], F32, tag="kk")
                nc.sync.dma_start(kk[:], kk_d.ap())
                kv = big.tile([128, NT, D], F32, tag="kv")
                nc.sync.dma_start(kv[:], kv_d.ap().rearrange("(nt p) d -> p nt d", p=128))

                # QT [R, tok]
                qp_full = ps.tile([128, 512], F32, tag="mm", name="qp")
                qp = qp_full[:, :128]
                for dt in range(DT):
                    nc.tensor.matmul(qp[:], sc[:, dt, :], xn[:, dt, :],
                                     start=(dt == 0), stop=(dt == DT - 1))
                q = work.tile([128, 128], F32, tag="q")
                nc.vector.tensor_copy(q[:], qp[:])

                # scores token-major [tok, NK] (scaled)
                s = work.tile([128, NK], F32, tag="s")
                for c2 in range(2):
                    sp = ps.tile([128, 512], F32, tag="mm")
                    nc.tensor.matmul(sp[:], q[:], kk[:, c2 * 512:(c2 + 1) * 512])
                    nc.vector.tensor_scalar_mul(s[:, c2 * 512:(c2 + 1) * 512], sp[:],
                                                inv_sqrt_r)
                # scoresT [nk, tok] (scaled)
                st = work.tile([128, NT, 128], F32, tag="st")
                for nt in range(NT):
                    tp = ps1.tile([128, 128], F32, tag="aux")
                    nc.tensor.matmul(tp[:], kk[:, nt * 128:(nt + 1) * 128], q[:])
                    nc.vector.tensor_scalar_mul(st[:, nt, :], tp[:], inv_sqrt_r)

                # top-16 threshold per token row: tau strictly between 16th/17th
                m8a = work.tile([128, 8], F32, tag="m8a")
                m8b = work.tile([128, 8], F32, tag="m8b")
                m8c = work.tile([128, 8], F32, tag="m8c")
                s2 = work.tile([128, NK], F32, tag="s2")
                s3 = work.tile([128, NK], F32, tag="s3")
                nc.vector.max(m8a[:], s[:])
                nc.vector.match_replace(s2[:], m8a[:], s[:], NEG)
                nc.vector.max(m8b[:], s2[:])
                nc.vector.match_replace(s3[:], m8b[:], s2[:], NEG)
                nc.vector.max(m8c[:], s3[:])
                tau = work.tile([128, 1], F32, tag="tau")
                nc.vector.tensor_add(tau[:], m8b[:, 7:8], m8c[:, 0:1])
                nc.vector.tensor_scalar_mul(tau[:], tau[:], 0.5)
                negm = work.tile([128, 1], F32, tag="negm")
                nc.vector.tensor_scalar_mul(negm[:], m8a[:, 0:1], -1.0)

                # Z per token from token-major layout (per-partition stats)
                etok = work.tile([128, NK], F32, tag="etok")
                nc.scalar.activation(etok[:], s[:], mybir.ActivationFunctionType.Exp,
                                     bias=negm[:])
                msk = work.tile([128, NK], F32, tag="msk")
                nc.vector.tensor_scalar(msk[:], s[:], tau[:], scalar2=None,
                                        op0=mybir.AluOpType.is_gt)
                nc.vector.tensor_mul(etok[:], etok[:], msk[:])
                z = work.tile([128, 1], F32, tag="z")
                nc.vector.reduce_sum(z[:], etok[:], axis=mybir.AxisListType.X)
                zr = work.tile([128, 1], F32, tag="zr")
                nc.vector.reciprocal(zr[:], z[:])

                # stats broadcast to T-layout (tok along free dim):
                # transpose [128,1] -> [1,128], then ones-column outer product
                idn = big.tile([128, 128], F32, tag="idn")
                make_identity(nc, idn[:])
                ones_row = work.tile([1, 128], F32, tag="ones_row")
                nc.vector.memset(ones_row[:], 1.0)

                def bcast_T(col, nm):
                    rp_f = ps1.tile([128, 128], F32, tag="aux", name=f"{nm}_rp")
                    rp = rp_f[:1, :]
                    nc.tensor.transpose(rp[:], col[:], idn[:])
                    row = work.tile([1, 128], F32, tag=f"{nm}_row", name=f"{nm}_row")
                    nc.vector.tensor_copy(row[:], rp[:])
                    bp = ps1.tile([128, 128], F32, tag="aux", name=f"{nm}_bp")
                    nc.tensor.matmul(bp[:], ones_row[:], row[:])
                    bc = work.tile([128, 128], F32, tag=f"{nm}_bc", name=f"{nm}_bc")
                    nc.vector.tensor_copy(bc[:], bp[:])
                    return bc

                negmT = bcast_T(negm, "negm")
                tauT = bcast_T(tau, "tau")

                # masked exp in T-layout, then WV matmul
                et = work.tile([128, NT, 128], F32, tag="et")
                for nt in range(NT):
                    nc.vector.tensor_add(et[:, nt, :], st[:, nt, :], negmT[:])
                    nc.scalar.activation(et[:, nt, :], et[:, nt, :],
                                         mybir.ActivationFunctionType.Exp)
                    mk = work.tile([128, 128], F32, tag="mk")
                    nc.vector.tensor_tensor(mk[:], st[:, nt, :], tauT[:],
                                            op=mybir.AluOpType.is_gt)
                    nc.vector.tensor_mul(et[:, nt, :], et[:, nt, :], mk[:])

                out = work.tile([128, D], F32, tag="out")
                for c2 in range(2):
                    op = ps.tile([128, 512], F32, tag="mm")
                    for nt in range(NT):
                        nc.tensor.matmul(op[:], et[:, nt, :],
                                         kv[:, nt, c2 * 512:(c2 + 1) * 512],
                                         start=(nt == 0), stop=(nt == NT - 1))
                    nc.vector.tensor_scalar_mul(out[:, c2 * 512:(c2 + 1) * 512],
                                                op[:], zr[:])
                nc.sync.dma_start(mo_d.ap(), out[:])

            if n_iter == 1:
                body(0)
            else:
                with tc.For_i(0, n_iter, 1) as it:
                    body(it)
    nc.compile()
    return nc


def _build_D(n_iter: int = 1):
    """lm_head, vocab-sharded. Inputs: xfT [D, T]; hwT [D, VSL].
    Output: lo [T, VSL]."""
    nc = bacc.Bacc("TRN2", target_bir_lowering=False, debug=False,
                   num_devices=N_CORES)
    xf_d = nc.dram_tensor("xfT", [D, T], F32, kind="ExternalInput")
    hw_d = nc.dram_tensor("hwT", [D, VSL], F32, kind="ExternalInput")
    lo_d = nc.dram_tensor("lo", [T, VSL], F32, kind="ExternalOutput")
    NVC = VSL // 512  # 8 chunks

    with tile.TileContext(nc) as tc:
        with (
            tc.tile_pool(name="big", bufs=1) as big,
            tc.tile_pool(name="wpool", bufs=3) as wpool,
            tc.tile_pool(name="opool", bufs=4) as opool,
            tc.tile_pool(name="ps", bufs=8, space="PSUM") as ps,
        ):
            def body(_it):
                xf = big.tile([128, DT, T], F32, tag="xf")
                nc.sync.dma_start(xf[:], xf_d.ap().rearrange("(dt p) t -> p dt t", p=128))
                for vc in range(NVC):
                    hw = wpool.tile([128, DT, 512], F32, tag="hw")
                    nc.sync.dma_start(
                        hw[:], hw_d.ap()[:, vc * 512:(vc + 1) * 512]
                        .rearrange("(dt p) v -> p dt v", p=128))
                    for tt in range(DT):
                        pp = ps.tile([128, 512], F32, tag="pp")
                        for dt in range(DT):
                            nc.tensor.matmul(pp[:], xf[:, dt, tt * 128:(tt + 1) * 128],
                                             hw[:, dt, :],
                                             start=(dt == 0), stop=(dt == DT - 1))
                        ot = opool.tile([128, 512], F32, tag="ot")
                        nc.vector.tensor_copy(ot[:], pp[:])
                        nc.sync.dma_start(
                            lo_d.ap()[tt * 128:(tt + 1) * 128,
                                      vc * 512:(vc + 1) * 512], ot[:])

            if n_iter == 1:
                body(0)
            else:
                with tc.For_i(0, n_iter, 1) as it:
                    body(it)
    nc.compile()
    return nc


_PROGS = {}


def _prog(name, n_iter=1):
    key = (name, n_iter)
    if key not in _PROGS:
        _PROGS[key] = {"A": _build_A, "C": _build_C, "D": _build_D}[name](n_iter)
    return _PROGS[key]


# ---------------------------------------------------------------- host-side math


def _ln(x, w, b):
    m = x.mean(-1, keepdims=True, dtype=np.float32)
    v = ((x - m) ** 2).mean(-1, keepdims=True, dtype=np.float32)
    return ((x - m) / np.sqrt(v + np.float32(1e-5)) * w + b).astype(np.float32)


def _softmax(x, axis=-1):
    m = x.max(axis=axis, keepdims=True)
    e = np.exp(x - m)
    return e / e.sum(axis=axis, keepdims=True)


def _nw(xn, A, Bm, Wimp, Wr):
    """SSM scan + routing -> neuron weights [B, NC] (host, fp32)."""
    u = xn @ Bm                       # [B,S,SD]
    h = np.zeros((xn.shape[0], A.shape[0]), np.float32)
    for t in range(xn.shape[1]):
        h = h @ A + u[:, t]
    h_proj = h @ Wimp.T               # [B, D]
    imp = _softmax(np.einsum('bsd,bd->bs', xn, h_proj), axis=-1)
    pref = _softmax(xn @ Wr.T, axis=-1)
    nw = np.einsum('bs,bsn->bn', imp, pref)
    return (nw / (nw.sum(-1, keepdims=True) + np.float32(1e-8))).astype(np.float32)


def _pack_T(x):
    """[B,S,D] -> d-major [D, B*S] fp32 contiguous."""
    return np.ascontiguousarray(
        np.concatenate([x[b].T for b in range(B)], axis=1), dtype=np.float32)


_run_ncores = list(range(N_CORES))
_LAST_MAPS = {}


def _run(name, in_maps):
    _LAST_MAPS[name] = in_maps
    res = run_bass_kernel_spmd(_prog(name), in_maps, core_ids=_run_ncores)
    return res.results


def kernel(**inputs) -> np.ndarray:
    inp = {k: np.asarray(v) for k, v in inputs.items()}
    ids = inp['input_ids'].astype(np.int64)
    comp_f = inp['compress_neurons'].reshape(NC, -1).astype(np.float32)
    tri = np.triu(np.ones((128, 128), np.float32))
    kKT = np.ascontiguousarray(inp['knowledge_K'].T, dtype=np.float32)
    kV = np.ascontiguousarray(inp['knowledge_V'], dtype=np.float32)

    x = (inp['tok_emb'][ids] + inp['pos_emb'][None, :ids.shape[1]]).astype(np.float32)

    for l in range(L):
        # ---- circuit (device program A, head-sharded) ----
        xn = _ln(x, inp['ln1_w'][l], inp['ln1_b'][l])
        nw = _nw(xn, inp['a_A'][l], inp['a_B'][l], inp['a_imp'][l], inp['a_router'][l])
        sc = (nw @ comp_f).reshape(B, D, R)
        eq = (nw @ inp['eQ'][l].reshape(NC, -1).astype(np.float32)).reshape(B, R, D)
        ek = (nw @ inp['eK'][l].reshape(NC, -1).astype(np.float32)).reshape(B, R, D)
        ev = (nw @ inp['eV'][l].reshape(NC, -1).astype(np.float32)).reshape(B, R, D)
        woT = np.ascontiguousarray(inp['o_w'][l].T, dtype=np.float32)
        xnT = _pack_T(xn)
        in_maps = []
        for c in range(N_CORES):
            sl = slice(128 * c, 128 * (c + 1))
            in_maps.append({
                "xnT": xnT,
                "sc": np.ascontiguousarray(sc, dtype=np.float32),
                "eqs": np.ascontiguousarray(eq[:, :, sl]),
                "eks": np.ascontiguousarray(ek[:, :, sl]),
                "evs": np.ascontiguousarray(ev[:, :, sl]),
                "woT": np.ascontiguousarray(woT[sl, :]),
                "tri": tri,
            })
        res = _run("A", in_maps)
        circT = res[0]["part"]
        for c in range(1, N_CORES):
            circT = circT + res[c]["part"]
        x = x + circT.transpose(0, 2, 1)

        # ---- memory (device program C, token-sharded) ----
        xn = _ln(x, inp['ln2_w'][l], inp['ln2_b'][l])
        nw = _nw(xn, inp['m_A'][l], inp['m_B'][l], inp['m_imp'][l], inp['m_router'][l])
        sc = (nw @ comp_f).reshape(B, D, R)
        in_maps = []
        for c in range(N_CORES):
            bc, s0 = c // 4, 128 * (c % 4)
            in_maps.append({
                "xnTs": np.ascontiguousarray(xn[bc, s0:s0 + 128, :].T),
                "scb": np.ascontiguousarray(sc[bc]),
                "kKT": kKT,
                "kV": kV,
            })
        res = _run("C", in_maps)
        mo = np.empty((B, S, D), np.float32)
        for c in range(N_CORES):
            bc, s0 = c // 4, 128 * (c % 4)
            mo[bc, s0:s0 + 128] = res[c]["mo"]
        x = x + mo

    # ---- lm_head (device program D, vocab-sharded) ----
    xf = _ln(x, inp['lnf_w'], inp['lnf_b'])
    xfT = _pack_T(xf)
    hwT = np.zeros((D, VP), np.float32)
    hwT[:, :V] = inp['head_w'].astype(np.float32).T
    in_maps = [{"xfT": xfT,
                "hwT": np.ascontiguousarray(hwT[:, VSL * c:VSL * (c + 1)])}
               for c in range(N_CORES)]
    res = _run("D", in_maps)
    logits = np.concatenate([res[c]["lo"] for c in range(N_CORES)], axis=1)
    return logits[:, :V].reshape(B, S, V)


# revision 13
# speedup vs baseline: 9.9471x; 9.9471x over previous
"""Trainium2 Bass kernel for nn_DAWN_41549513621652.

Strategy (8 NeuronCores, single chip, no cross-core collectives):
  The model's heavy compute is dense matmul (attention, Wo, memory WV,
  lm_head). The glue (layernorm, the 512-step SSM scan, routing softmax,
  and the DMA-bound neuron-pool contractions nw@{comp,EQ,EK,EV}) is tiny
  FLOP-wise and runs on host between device launches; host also performs
  the cross-core reductions (summing Wo partials) so the device programs
  need no collectives.

  5 device launches per call:
    A (x2): circuit module, head-sharded — core c owns heads {2c, 2c+1}
            for both batch elements; outputs per-core Wo partials.
    C (x2): memory module, token-sharded — core c owns 128 tokens
            (b=c//4, s in [128*(c%4), ...)); exact top-16 via DVE
            max8/match_replace threshold, dense masked-softmax, PE WV.
    D (x1): lm_head, vocab-sharded — core c owns a 4096-wide slice of the
            zero-padded 32768 vocab.

  Everything is fp32: the memory module's top-16 selection has score gaps
  down to 7e-9, so any lower precision upstream flips selections vs the
  reference.
"""

import numpy as np

import concourse.bass as bass
import concourse.bacc as bacc
import concourse.mybir as mybir
import concourse.tile as tile
from concourse.bass_utils import run_bass_kernel_spmd
from concourse.masks import make_identity

F32 = mybir.dt.float32
F32R = mybir.dt.float32r


def _mmr(nc, out, lhsT, rhs, **kw):
    """float32r matmul (operand tiles are already float32r-typed)."""
    nc.tensor.matmul(out, lhsT, rhs, **kw)

# model dims (hardcoded per problem spec)
L, D, H, R, NC, NK, KK, SD, V, B, S = 2, 1024, 16, 128, 64, 1024, 16, 64, 32000, 2, 512
DH = D // H          # 64
T = B * S            # 1024
N_CORES = 8
VP = 32768           # padded vocab
VSL = VP // N_CORES  # 4096 per-core vocab slice
DT = D // 128        # 8 d-tiles
NEG = -1e30


# ---------------------------------------------------------------- device programs


def _build_A(n_iter: int = 1):
    """Circuit module. Per-core inputs:
      xnT  [D, T]    d-major normalized activations (cols = b*S+s)
      sc   [B, D, R] dynamic compress basis (host: nw@comp)
      eqs/eks/evs [B, R, 128]  expansion slices for this core's 2 heads
      woT  [128, D]  o_w.T rows for this core's d_in slice
      tri  [128, 128] upper-tri (incl diag) causal mask for scoresT layout
    Output:
      part [B, D, S] Wo partial, d-major
    """
    nc = bacc.Bacc("TRN2", target_bir_lowering=False, debug=False,
                   num_devices=N_CORES)
    xnT_d = nc.dram_tensor("xnT", [D, T], F32R, kind="ExternalInput")
    sc_d = nc.dram_tensor("sc", [B, D, R], F32R, kind="ExternalInput")
    eqs_d = nc.dram_tensor("eqs", [B, R, 128], F32R, kind="ExternalInput")
    eks_d = nc.dram_tensor("eks", [B, R, 128], F32R, kind="ExternalInput")
    evs_d = nc.dram_tensor("evs", [B, R, 128], F32R, kind="ExternalInput")
    woT_d = nc.dram_tensor("woT", [128, D], F32R, kind="ExternalInput")
    tri_d = nc.dram_tensor("tri", [128, 128], F32R, kind="ExternalInput")
    part_d = nc.dram_tensor("part", [B, D, S], F32, kind="ExternalOutput")

    with tile.TileContext(nc) as tc:
        with (
            tc.tile_pool(name="big", bufs=1) as big,
            tc.tile_pool(name="work", bufs=2) as work,
            tc.tile_pool(name="small", bufs=2) as small,
            tc.tile_pool(name="ps", bufs=2, space="PSUM") as ps,
            tc.tile_pool(name="ps1", bufs=2, space="PSUM") as ps1,
            tc.tile_pool(name="out", bufs=3) as outp,
        ):
            def body(_it):
                xn = big.tile([128, DT, T], F32R, tag="xn")
                nc.sync.dma_start(xn[:], xnT_d.ap().rearrange("(dt p) t -> p dt t", p=128))
                sc = big.tile([128, B, DT, R], F32R, tag="sc")
                nc.sync.dma_start(sc[:], sc_d.ap().rearrange("b (dt p) r -> p b dt r", p=128))
                eq = big.tile([128, B, 128], F32R, tag="eq")
                ek = big.tile([128, B, 128], F32R, tag="ek")
                ev = big.tile([128, B, 128], F32R, tag="ev")
                nc.sync.dma_start(eq[:], eqs_d.ap().rearrange("b r e -> r b e"))
                nc.sync.dma_start(ek[:], eks_d.ap().rearrange("b r e -> r b e"))
                nc.sync.dma_start(ev[:], evs_d.ap().rearrange("b r e -> r b e"))
                wo = big.tile([128, D], F32R, tag="wo")
                nc.sync.dma_start(wo[:], woT_d.ap())
                tri = big.tile([128, 128], F32R, tag="tri")
                nc.sync.dma_start(tri[:], tri_d.ap())
                ones = big.tile([128, 1], F32R, tag="ones")
                nc.vector.memset(ones[:].bitcast(F32), 1.0)
                ones_row = big.tile([1, 64], F32, tag="ones_row")
                nc.vector.memset(ones_row[:], 1.0)

                # hT[b] [R=128, S] = sc[b].T @ xnT[b]
                h = big.tile([128, B, S], F32R, tag="h")
                for b in range(B):
                    hp = ps.tile([128, S], F32, tag="mm")
                    for dt in range(DT):
                        _mmr(nc, hp[:], sc[:, b, dt, :], xn[:, dt, b * S:(b + 1) * S],
                                         start=(dt == 0), stop=(dt == DT - 1))
                    nc.vector.tensor_copy(h[:, b, :], hp[:])

                # QT/KT [128(dh2), B, S]; V token-major [128(tok), B, 4, 128(dh2)]
                qt = big.tile([128, B, S], F32R, tag="qt")
                kt_ = big.tile([128, B, S], F32R, tag="kt")
                vt = big.tile([128, B, 4, 128], F32R, tag="vt")
                for b in range(B):
                    qp = ps.tile([128, S], F32, tag="mm")
                    _mmr(nc, qp[:], eq[:, b, :], h[:, b, :])
                    nc.vector.tensor_copy(qt[:, b, :], qp[:])
                    kp = ps.tile([128, S], F32, tag="mm")
                    _mmr(nc, kp[:], ek[:, b, :], h[:, b, :])
                    nc.vector.tensor_copy(kt_[:, b, :], kp[:])
                    for st in range(4):
                        vp = ps1.tile([128, 512], F32, tag="aux1", name="vp")[:, :128]
                        _mmr(nc, vp[:], h[:, b, st * 128:(st + 1) * 128], ev[:, b, :])
                        nc.vector.tensor_copy(vt[:, b, st, :], vp[:])

                # attention per (b, head-in-core)
                att = big.tile([128, B, S], F32R, tag="att")  # [d_in(2*64), b, q]
                for b in range(B):
                    for hh in range(2):
                        p0 = 64 * hh
                        et = work.tile([128, 4, S], F32R, tag="et")
                        for kt in range(4):
                            q0 = 128 * kt
                            sp = ps.tile([128, S], F32, tag="sp")
                            _mmr(nc, 
                                sp[:, q0:S],
                                kt_[p0:p0 + 64, b, kt * 128:(kt + 1) * 128],
                                qt[p0:p0 + 64, b, q0:S])
                            # e = exp(s / sqrt(DH)), causal-masked on the diagonal block
                            nc.scalar.activation(et[:, kt, q0:S], sp[:, q0:S],
                                                 mybir.ActivationFunctionType.Exp,
                                                 scale=float(1.0 / np.sqrt(DH)))
                            nc.vector.tensor_mul(et[:, kt, q0:q0 + 128],
                                                 et[:, kt, q0:q0 + 128], tri[:])
                        # Z[q] = sum_k e[k,q] via ones-matmul; then 1/Z
                        zp = ps1.tile([128, S], F32, tag="aux1", name="zp")[:1, :]
                        for kt in range(4):
                            _mmr(nc, zp[:, 128 * kt:S], ones[:],
                                             et[:, kt, 128 * kt:S],
                                             start=(kt == 0), stop=(kt == 3))
                        zr = small.tile([1, S], F32, tag="zr")
                        nc.vector.reciprocal(zr[:], zp[:])
                        zbp = ps1.tile([128, S], F32, tag="aux1", name="zbp")[:64, :]
                        _mmr(nc, zbp[:], ones_row[:], zr[:])
                        zb = small.tile([64, S], F32R, tag="zb")
                        nc.vector.tensor_copy(zb[:], zbp[:])
                        # out_hT [64(dh), S] = sum_k V[k,dh].T-form @ e[k,q]
                        op_full = ps.tile([128, S], F32, tag="op", name="op")
                        op = op_full[:64, :]
                        for kt in range(4):
                            _mmr(nc, op[:, 128 * kt:S],
                                             vt[:, b, kt, p0:p0 + 64],
                                             et[:, kt, 128 * kt:S],
                                             start=(kt == 0), stop=(kt == 3))
                        nc.vector.tensor_mul(att[p0:p0 + 64, b, :], op[:], zb[:])

                # Wo partial: part[b].T [d_out, S] = woT.T @ att[b]
                for b in range(B):
                    for mt in range(DT):
                        wp = ps.tile([128, S], F32, tag="mm")
                        _mmr(nc, wp[:], wo[:, mt * 128:(mt + 1) * 128], att[:, b, :])
                        ot = outp.tile([128, S], F32, tag="ot")
                        nc.vector.tensor_copy(ot[:], wp[:])
                        nc.sync.dma_start(
                            part_d.ap()[b, mt * 128:(mt + 1) * 128, :], ot[:])

            if n_iter == 1:
                body(0)
            else:
                with tc.For_i(0, n_iter, 1) as it:
                    body(it)
    nc.compile()
    return nc


def _build_C(n_iter: int = 1):
    """Memory module, token-sharded (128 tokens per core). Inputs:
      xnTs [D, 128]  d-major xn columns for this core's tokens
      scb  [D, R]    compress basis for this core's batch element
      kKT  [R, NK]   knowledge_K.T
      kV   [NK, D]
    Output: mo [128, D] memory output rows for this core's tokens."""
    nc = bacc.Bacc("TRN2", target_bir_lowering=False, debug=False,
                   num_devices=N_CORES)
    xn_d = nc.dram_tensor("xnTs", [D, 128], F32, kind="ExternalInput")
    sc_d = nc.dram_tensor("scb", [D, R], F32, kind="ExternalInput")
    kk_d = nc.dram_tensor("kKT", [R, NK], F32, kind="ExternalInput")
    kv_d = nc.dram_tensor("kV", [NK, D], F32R, kind="ExternalInput")
    mo_d = nc.dram_tensor("mo", [128, D], F32, kind="ExternalOutput")
    NT = NK // 128  # 8
    inv_sqrt_r = float(1.0 / np.sqrt(R))

    with tile.TileContext(nc) as tc:
        with (
            tc.tile_pool(name="big", bufs=1) as big,
            tc.tile_pool(name="work", bufs=2) as work,
            tc.tile_pool(name="ps", bufs=2, space="PSUM") as ps,
            tc.tile_pool(name="ps1", bufs=2, space="PSUM") as ps1,
        ):
            def body(_it):
                xn = big.tile([128, DT, 128], F32, tag="xn")
                nc.sync.dma_start(xn[:], xn_d.ap().rearrange("(dt p) t -> p dt t", p=128))
                sc = big.tile([128, DT, R], F32, tag="sc")
                nc.sync.dma_start(sc[:], sc_d.ap().rearrange("(dt p) r -> p dt r", p=128))
                kk = big.tile([128, NK], F32, tag="kk")
                nc.sync.dma_start(kk[:], kk_d.ap())
                kv = big.tile([128, NT, D], F32R, tag="kv")
                nc.sync.dma_start(kv[:], kv_d.ap().rearrange("(nt p) d -> p nt d", p=128))

                # QT [R, tok]
                qp_full = ps.tile([128, 512], F32, tag="mm", name="qp")
                qp = qp_full[:, :128]
                for dt in range(DT):
                    nc.tensor.matmul(qp[:], sc[:, dt, :], xn[:, dt, :],
                                     start=(dt == 0), stop=(dt == DT - 1))
                q = work.tile([128, 128], F32, tag="q")
                nc.vector.tensor_copy(q[:], qp[:])

                # scores token-major [tok, NK] (scaled)
                s = work.tile([128, NK], F32, tag="s")
                for c2 in range(2):
                    sp = ps.tile([128, 512], F32, tag="mm")
                    nc.tensor.matmul(sp[:], q[:], kk[:, c2 * 512:(c2 + 1) * 512])
                    nc.vector.tensor_scalar_mul(s[:, c2 * 512:(c2 + 1) * 512], sp[:],
                                                inv_sqrt_r)
                # scoresT [nk, tok] (scaled)
                st = work.tile([128, NT, 128], F32, tag="st")
                for nt in range(NT):
                    tp = ps1.tile([128, 128], F32, tag="aux")
                    nc.tensor.matmul(tp[:], kk[:, nt * 128:(nt + 1) * 128], q[:])
                    nc.vector.tensor_scalar_mul(st[:, nt, :], tp[:], inv_sqrt_r)

                # top-16 threshold per token row: tau strictly between 16th/17th
                m8a = work.tile([128, 8], F32, tag="m8a")
                m8b = work.tile([128, 8], F32, tag="m8b")
                m8c = work.tile([128, 8], F32, tag="m8c")
                s2 = work.tile([128, NK], F32, tag="s2")
                s3 = work.tile([128, NK], F32, tag="s3")
                nc.vector.max(m8a[:], s[:])
                nc.vector.match_replace(s2[:], m8a[:], s[:], NEG)
                nc.vector.max(m8b[:], s2[:])
                nc.vector.match_replace(s3[:], m8b[:], s2[:], NEG)
                nc.vector.max(m8c[:], s3[:])
                tau = work.tile([128, 1], F32, tag="tau")
                nc.vector.tensor_add(tau[:], m8b[:, 7:8], m8c[:, 0:1])
                nc.vector.tensor_scalar_mul(tau[:], tau[:], 0.5)
                negm = work.tile([128, 1], F32, tag="negm")
                nc.vector.tensor_scalar_mul(negm[:], m8a[:, 0:1], -1.0)

                # Z per token from token-major layout (per-partition stats)
                etok = work.tile([128, NK], F32, tag="etok")
                nc.scalar.activation(etok[:], s[:], mybir.ActivationFunctionType.Exp,
                                     bias=negm[:])
                msk = work.tile([128, NK], F32, tag="msk")
                nc.vector.tensor_scalar(msk[:], s[:], tau[:], scalar2=None,
                                        op0=mybir.AluOpType.is_gt)
                nc.vector.tensor_mul(etok[:], etok[:], msk[:])
                z = work.tile([128, 1], F32, tag="z")
                nc.vector.reduce_sum(z[:], etok[:], axis=mybir.AxisListType.X)
                zr = work.tile([128, 1], F32, tag="zr")
                nc.vector.reciprocal(zr[:], z[:])

                # stats broadcast to T-layout (tok along free dim):
                # transpose [128,1] -> [1,128], then ones-column outer product
                idn = big.tile([128, 128], F32, tag="idn")
                make_identity(nc, idn[:])
                ones_row = work.tile([1, 128], F32, tag="ones_row")
                nc.vector.memset(ones_row[:], 1.0)

                def bcast_T(col, nm):
                    rp_f = ps1.tile([128, 128], F32, tag="aux", name=f"{nm}_rp")
                    rp = rp_f[:1, :]
                    nc.tensor.transpose(rp[:], col[:], idn[:])
                    row = work.tile([1, 128], F32, tag=f"{nm}_row", name=f"{nm}_row")
                    nc.vector.tensor_copy(row[:], rp[:])
                    bp = ps1.tile([128, 128], F32, tag="aux", name=f"{nm}_bp")
                    nc.tensor.matmul(bp[:], ones_row[:], row[:])
                    bc = work.tile([128, 128], F32, tag=f"{nm}_bc", name=f"{nm}_bc")
                    nc.vector.tensor_copy(bc[:], bp[:])
                    return bc

                negmT = bcast_T(negm, "negm")
                tauT = bcast_T(tau, "tau")

                # masked exp in T-layout, then WV matmul
                et = work.tile([128, NT, 128], F32R, tag="et")
                for nt in range(NT):
                    nc.vector.tensor_add(et[:, nt, :], st[:, nt, :], negmT[:])
                    nc.scalar.activation(et[:, nt, :], et[:, nt, :],
                                         mybir.ActivationFunctionType.Exp)
                    mk = work.tile([128, 128], F32, tag="mk")
                    nc.vector.tensor_tensor(mk[:], st[:, nt, :], tauT[:],
                                            op=mybir.AluOpType.is_gt)
                    nc.vector.tensor_mul(et[:, nt, :], et[:, nt, :], mk[:])

                out = work.tile([128, D], F32, tag="out")
                for c2 in range(2):
                    op = ps.tile([128, 512], F32, tag="mm")
                    for nt in range(NT):
                        _mmr(nc, op[:], et[:, nt, :],
                             kv[:, nt, c2 * 512:(c2 + 1) * 512],
                             start=(nt == 0), stop=(nt == NT - 1))
                    nc.vector.tensor_scalar_mul(out[:, c2 * 512:(c2 + 1) * 512],
                                                op[:], zr[:])
                nc.sync.dma_start(mo_d.ap(), out[:])

            if n_iter == 1:
                body(0)
            else:
                with tc.For_i(0, n_iter, 1) as it:
                    body(it)
    nc.compile()
    return nc


def _build_D(n_iter: int = 1):
    """lm_head, vocab-sharded. Inputs: xfT [D, T]; hwT [D, VSL].
    Output: lo [T, VSL]."""
    nc = bacc.Bacc("TRN2", target_bir_lowering=False, debug=False,
                   num_devices=N_CORES)
    xf_d = nc.dram_tensor("xfT", [D, T], F32R, kind="ExternalInput")
    hw_d = nc.dram_tensor("hwT", [D, VSL], F32R, kind="ExternalInput")
    lo_d = nc.dram_tensor("lo", [T, VSL], F32, kind="ExternalOutput")
    NVC = VSL // 512  # 8 chunks

    with tile.TileContext(nc) as tc:
        with (
            tc.tile_pool(name="big", bufs=1) as big,
            tc.tile_pool(name="wpool", bufs=3) as wpool,
            tc.tile_pool(name="opool", bufs=4) as opool,
            tc.tile_pool(name="ps", bufs=8, space="PSUM") as ps,
        ):
            def body(_it):
                xf = big.tile([128, DT, T], F32R, tag="xf")
                nc.sync.dma_start(xf[:], xf_d.ap().rearrange("(dt p) t -> p dt t", p=128))
                for vc in range(NVC):
                    hw = wpool.tile([128, DT, 512], F32R, tag="hw")
                    nc.sync.dma_start(
                        hw[:], hw_d.ap()[:, vc * 512:(vc + 1) * 512]
                        .rearrange("(dt p) v -> p dt v", p=128))
                    for tt in range(DT):
                        pp = ps.tile([128, 512], F32, tag="pp")
                        for dt in range(DT):
                            _mmr(nc, pp[:], xf[:, dt, tt * 128:(tt + 1) * 128],
                                             hw[:, dt, :],
                                             start=(dt == 0), stop=(dt == DT - 1))
                        ot = opool.tile([128, 512], F32, tag="ot")
                        nc.vector.tensor_copy(ot[:], pp[:])
                        nc.sync.dma_start(
                            lo_d.ap()[tt * 128:(tt + 1) * 128,
                                      vc * 512:(vc + 1) * 512], ot[:])

            if n_iter == 1:
                body(0)
            else:
                with tc.For_i(0, n_iter, 1) as it:
                    body(it)
    nc.compile()
    return nc


_PROGS = {}


def _prog(name, n_iter=1):
    key = (name, n_iter)
    if key not in _PROGS:
        _PROGS[key] = {"A": _build_A, "C": _build_C, "D": _build_D}[name](n_iter)
    return _PROGS[key]


# ---------------------------------------------------------------- host-side math


def _ln(x, w, b):
    m = x.mean(-1, keepdims=True, dtype=np.float32)
    v = ((x - m) ** 2).mean(-1, keepdims=True, dtype=np.float32)
    return ((x - m) / np.sqrt(v + np.float32(1e-5)) * w + b).astype(np.float32)


def _softmax(x, axis=-1):
    m = x.max(axis=axis, keepdims=True)
    e = np.exp(x - m)
    return e / e.sum(axis=axis, keepdims=True)


def _nw(xn, A, Bm, Wimp, Wr):
    """SSM scan + routing -> neuron weights [B, NC] (host, fp32)."""
    u = xn @ Bm                       # [B,S,SD]
    h = np.zeros((xn.shape[0], A.shape[0]), np.float32)
    for t in range(xn.shape[1]):
        h = h @ A + u[:, t]
    h_proj = h @ Wimp.T               # [B, D]
    imp = _softmax(np.einsum('bsd,bd->bs', xn, h_proj), axis=-1)
    pref = _softmax(xn @ Wr.T, axis=-1)
    nw = np.einsum('bs,bsn->bn', imp, pref)
    return (nw / (nw.sum(-1, keepdims=True) + np.float32(1e-8))).astype(np.float32)


def _pack_T(x):
    """[B,S,D] -> d-major [D, B*S] fp32 contiguous."""
    return np.ascontiguousarray(
        np.concatenate([x[b].T for b in range(B)], axis=1), dtype=np.float32)


_run_ncores = list(range(N_CORES))
_LAST_MAPS = {}


def _run(name, in_maps):
    _LAST_MAPS[name] = in_maps
    res = run_bass_kernel_spmd(_prog(name), in_maps, core_ids=_run_ncores)
    return res.results


def kernel(**inputs) -> np.ndarray:
    inp = {k: np.asarray(v) for k, v in inputs.items()}
    ids = inp['input_ids'].astype(np.int64)
    comp_f = inp['compress_neurons'].reshape(NC, -1).astype(np.float32)
    tri = np.triu(np.ones((128, 128), np.float32))
    kKT = np.ascontiguousarray(inp['knowledge_K'].T, dtype=np.float32)
    kV = np.ascontiguousarray(inp['knowledge_V'], dtype=np.float32)

    x = (inp['tok_emb'][ids] + inp['pos_emb'][None, :ids.shape[1]]).astype(np.float32)

    for l in range(L):
        # ---- circuit (device program A, head-sharded) ----
        xn = _ln(x, inp['ln1_w'][l], inp['ln1_b'][l])
        nw = _nw(xn, inp['a_A'][l], inp['a_B'][l], inp['a_imp'][l], inp['a_router'][l])
        sc = (nw @ comp_f).reshape(B, D, R)
        eq = (nw @ inp['eQ'][l].reshape(NC, -1).astype(np.float32)).reshape(B, R, D)
        ek = (nw @ inp['eK'][l].reshape(NC, -1).astype(np.float32)).reshape(B, R, D)
        ev = (nw @ inp['eV'][l].reshape(NC, -1).astype(np.float32)).reshape(B, R, D)
        woT = np.ascontiguousarray(inp['o_w'][l].T, dtype=np.float32)
        xnT = _pack_T(xn)
        in_maps = []
        for c in range(N_CORES):
            sl = slice(128 * c, 128 * (c + 1))
            in_maps.append({
                "xnT": xnT,
                "sc": np.ascontiguousarray(sc, dtype=np.float32),
                "eqs": np.ascontiguousarray(eq[:, :, sl]),
                "eks": np.ascontiguousarray(ek[:, :, sl]),
                "evs": np.ascontiguousarray(ev[:, :, sl]),
                "woT": np.ascontiguousarray(woT[sl, :]),
                "tri": tri,
            })
        res = _run("A", in_maps)
        circT = res[0]["part"]
        for c in range(1, N_CORES):
            circT = circT + res[c]["part"]
        x = x + circT.transpose(0, 2, 1)

        # ---- memory (device program C, token-sharded) ----
        xn = _ln(x, inp['ln2_w'][l], inp['ln2_b'][l])
        nw = _nw(xn, inp['m_A'][l], inp['m_B'][l], inp['m_imp'][l], inp['m_router'][l])
        sc = (nw @ comp_f).reshape(B, D, R)
        in_maps = []
        for c in range(N_CORES):
            bc, s0 = c // 4, 128 * (c % 4)
            in_maps.append({
                "xnTs": np.ascontiguousarray(xn[bc, s0:s0 + 128, :].T),
                "scb": np.ascontiguousarray(sc[bc]),
                "kKT": kKT,
                "kV": kV,
            })
        res = _run("C", in_maps)
        mo = np.empty((B, S, D), np.float32)
        for c in range(N_CORES):
            bc, s0 = c // 4, 128 * (c % 4)
            mo[bc, s0:s0 + 128] = res[c]["mo"]
        x = x + mo

    # ---- lm_head (device program D, vocab-sharded) ----
    xf = _ln(x, inp['lnf_w'], inp['lnf_b'])
    xfT = _pack_T(xf)
    hwT = np.zeros((D, VP), np.float32)
    hwT[:, :V] = inp['head_w'].astype(np.float32).T
    in_maps = [{"xfT": xfT,
                "hwT": np.ascontiguousarray(hwT[:, VSL * c:VSL * (c + 1)])}
               for c in range(N_CORES)]
    res = _run("D", in_maps)
    logits = np.concatenate([res[c]["lo"] for c in range(N_CORES)], axis=1)
    return logits[:, :V].reshape(B, S, V)
